# revision 10
# baseline (speedup 1.0000x reference)
"""Bahdanau-attention GRU decoder step on 8 Trainium2 NeuronCores.

Data-parallel over batch for attention (16 batches/core, enc_output shipped
in transposed + natural layouts so every DMA is contiguous); the tiny GRU/fc
tail is unit-sharded across cores (weights sliced 8x on host) with two
AllGathers + one AllReduce so its weights prefetch fully into SBUF and the
tail stays off the critical path. Heavy matmuls run as float32r (fp32
storage, full-rate PE). Context matmuls are software-pipelined between the
score groups of the following quad to keep the PE HAM-warm.
"""

from contextlib import ExitStack

import numpy as np

import concourse.bacc as bacc
import concourse.bass as bass
import concourse.mybir as mybir
import concourse.tile as tile
from concourse.bass_utils import run_bass_kernel_spmd

N_CORES = 8
B, S, U, IN = 128, 1024, 1024, 64
BL = B // N_CORES  # batches per core
KC = U // 128      # contraction chunks
MC = U // 128      # dec-unit chunks
SBLK = 512         # moving-dim block over S
NSB = S // SBLK
QB = 4             # batches per softmax/context quad
USL = 3 * 128      # per-core gate-unit slice (z|r|h cols of this core's units)

MODE = "f32r"      # "f32" | "f32r" | "bf16"

F32 = mybir.dt.float32
AF = mybir.ActivationFunctionType
AX = mybir.AxisListType


def _dt():
    return {
        "f32": F32,
        "f32r": mybir.dt.float32r,
        "bf16": mybir.dt.bfloat16,
    }[MODE]


def build_nc():
    nc = bacc.Bacc(
        "TRN2", target_bir_lowering=False, debug=False, num_devices=N_CORES
    )
    DT = _dt()

    # ---- DRAM I/O ----
    encT = nc.dram_tensor("encT", [BL, U, S], DT, kind="ExternalInput").ap()
    encN = nc.dram_tensor("encN", [BL, S, U], DT, kind="ExternalInput").ap()
    W1 = nc.dram_tensor("W1", [U, U], DT, kind="ExternalInput").ap()
    W2 = nc.dram_tensor("W2", [U, U], DT, kind="ExternalInput").ap()
    hTq = nc.dram_tensor("hTq", [U, BL], DT, kind="ExternalInput").ap()
    hTfull = nc.dram_tensor("hTfull", [U, B], DT, kind="ExternalInput").ap()
    hf_s = nc.dram_tensor("hf_s", [128, B], F32, kind="ExternalInput").ap()
    Vr = nc.dram_tensor("Vr", [128, MC], DT, kind="ExternalInput").ap()
    b12r = nc.dram_tensor("b12r", [128, MC], F32, kind="ExternalInput").ap()
    eye128 = nc.dram_tensor("eye128", [128, 128], F32, kind="ExternalInput").ap()
    x2Tf = nc.dram_tensor("x2Tf", [IN, B], DT, kind="ExternalInput").ap()
    Wk_s = nc.dram_tensor("Wk_s", [U + IN, USL], DT, kind="ExternalInput").ap()
    Wr_s = nc.dram_tensor("Wr_s", [U, USL], DT, kind="ExternalInput").ap()
    bg_s = nc.dram_tensor("bg_s", [128, 3], F32, kind="ExternalInput").ap()
    Wfc_s = nc.dram_tensor("Wfc_s", [128, IN], DT, kind="ExternalInput").ap()
    bfcr = nc.dram_tensor("bfcr", [IN, 1], F32, kind="ExternalInput").ap()

    attnW = nc.dram_tensor("attnW", [BL, S], F32, kind="ExternalOutput").ap()
    stateT_s = nc.dram_tensor("stateT_s", [128, B], F32, kind="ExternalOutput").ap()
    outT = nc.dram_tensor("outT", [IN, B], F32, kind="ExternalOutput").ap()

    with tile.TileContext(nc) as tc, ExitStack() as ctx:
        cpool = ctx.enter_context(tc.tile_pool(name="const", bufs=1))
        w1p = ctx.enter_context(tc.tile_pool(name="w1", bufs=1))
        gcp = ctx.enter_context(tc.tile_pool(name="gconst", bufs=1))
        dramp = ctx.enter_context(tc.tile_pool(name="dram", bufs=1, space="DRAM"))

        # ---- W1 as per-m column blocks so the first matmul starts early ----
        W1r = W1.rearrange("(k p) u -> p k u", p=128)
        w1m = []
        for m in range(MC):
            t = w1p.tile([128, KC, 128], DT, tag=f"w1_{m}", name=f"w1m{m}")
            nc.sync.dma_start(t[:], W1r[:, :, m * 128:(m + 1) * 128])
            w1m.append(t)

        # ---- constants / small inputs ----
        eye_sb = cpool.tile([128, 128], F32, tag="eye")
        nc.sync.dma_start(eye_sb[:], eye128)
        vr_sb = cpool.tile([128, MC], DT, tag="vr")
        nc.sync.dma_start(vr_sb[:], Vr)
        b12_sb = cpool.tile([128, MC], F32, tag="b12")
        nc.sync.dma_start(b12_sb[:], b12r)
        hTq_sb = cpool.tile([128, KC, BL], DT, tag="hTq")
        nc.sync.dma_start(hTq_sb[:], hTq.rearrange("(k p) b -> p k b", p=128))
        qT_sb = cpool.tile([128, MC, BL], F32, tag="qT")
        attnT_sb = cpool.tile([128, KC, BL], DT, tag="attnT")
        ctx_sb = cpool.tile([BL, U], F32, tag="ctxrow")

        # ---- GRU weights (unit-sharded -> small; prefetch at start) ----
        wkA = gcp.tile([128, KC, USL], DT, tag="wkA")
        nc.sync.dma_start(
            wkA[:], Wk_s[0:U, :].rearrange("(k p) j -> p k j", p=128)
        )
        wkB = gcp.tile([IN, USL], DT, tag="wkB")
        nc.sync.dma_start(wkB[:], Wk_s[U:U + IN, :])
        wr_sb = gcp.tile([128, KC, USL], DT, tag="wr")
        nc.sync.dma_start(wr_sb[:], Wr_s.rearrange("(k p) j -> p k j", p=128))
        hTf_sb = gcp.tile([128, KC, B], DT, tag="hTfull")
        nc.sync.dma_start(hTf_sb[:], hTfull.rearrange("(k p) b -> p k b", p=128))
        hfs_sb = gcp.tile([128, B], F32, tag="hfs")
        nc.sync.dma_start(hfs_sb[:], hf_s)
        x2_sb = gcp.tile([IN, B], DT, tag="x2")
        nc.sync.dma_start(x2_sb[:], x2Tf)
        bgs_sb = gcp.tile([128, 3], F32, tag="bgs")
        nc.sync.dma_start(bgs_sb[:], bg_s)
        wfc_sb = gcp.tile([128, IN], DT, tag="wfc")
        nc.sync.dma_start(wfc_sb[:], Wfc_s)
        bfc_sb = gcp.tile([IN, 1], F32, tag="bfc")
        nc.sync.dma_start(bfc_sb[:], bfcr)

        # collective bounce buffers (internal DRAM)
        cgin = dramp.tile([BL, U], F32, tag="cgin")
        cgout = dramp.tile([B, U], F32, tag="cgout")
        rgin = dramp.tile([128, B], DT, tag="rgin")
        rgout = dramp.tile([U, B], DT, tag="rgout")
        fcin = dramp.tile([IN, B], F32, tag="fcin")
        fcout = dramp.tile([IN, B], F32, tag="fcout")

        # ---- phase 0: qT = (hidden @ W2 + b1 + b2)^T for own batches ----
        with tc.tile_pool(name="w2s", bufs=2) as w2p, \
             tc.tile_pool(name="ph0ps", bufs=8, space="PSUM") as pps:
            qps = [pps.tile([128, BL], F32, tag="qps", name=f"qps{m}")
                   for m in range(MC)]
            for k in range(KC):
                w2t = w2p.tile([128, U], DT, tag="w2t")
                nc.sync.dma_start(w2t[:], W2[k * 128:(k + 1) * 128, :])
                for m in range(MC):
                    nc.tensor.matmul(
                        qps[m][:],
                        lhsT=w2t[:, m * 128:(m + 1) * 128],
                        rhs=hTq_sb[:, k, :],
                        start=(k == 0),
                        stop=(k == KC - 1),
                    )
            for m in range(MC):
                nc.scalar.activation(
                    qT_sb[:, m, :], qps[m][:], AF.Identity,
                    bias=b12_sb[:, m:m + 1],
                )

        # ---- scores / softmax / context, software-pipelined per quad ----
        with tc.tile_pool(name="encTp", bufs=2) as etp, \
             tc.tile_pool(name="tf", bufs=12) as tfp, \
             tc.tile_pool(name="encNp", bufs=2) as enp, \
             tc.tile_pool(name="sm", bufs=2) as smp, \
             tc.tile_pool(name="featps", bufs=3, space="PSUM") as fps, \
             tc.tile_pool(name="scoreps", bufs=2, space="PSUM") as sps, \
             tc.tile_pool(name="ctxps", bufs=2, space="PSUM") as cps, \
             tc.tile_pool(name="tpps", bufs=1, space="PSUM") as tps:

            sq_tiles = {}
            pending_ctx = []  # batches whose context pass is ready to emit

            def emit_score_group(b, sb):
                q = b // QB
                if (b % QB, sb) == (0, 0):
                    sq_tiles[q] = smp.tile([QB, S], F32, tag="sq",
                                           name=f"sq{q}")
                sq = sq_tiles[q]
                et = etp.tile([128, KC, SBLK], DT, tag="et")
                src = encT[b].rearrange("(k p) s -> p k s", p=128)
                for k in range(KC):
                    nc.sync.dma_start(
                        et[:, k, :],
                        src[:, k, sb * SBLK:(sb + 1) * SBLK],
                    )
                tfs = []
                for m in range(MC):
                    fp = fps.tile([128, SBLK], F32, tag="fp")
                    for k in range(KC):
                        nc.tensor.matmul(
                            fp[:],
                            lhsT=w1m[m][:, k, :],
                            rhs=et[:, k, :],
                            start=(k == 0),
                            stop=(k == KC - 1),
                        )
                    tf = tfp.tile([128, SBLK], DT, tag="tf")
                    nc.scalar.activation(
                        tf[:], fp[:], AF.Tanh, bias=qT_sb[:, m, b:b + 1]
                    )
                    tfs.append(tf)
                sp = sps.tile([1, SBLK], F32, tag="sp")
                for m in range(MC):
                    nc.tensor.matmul(
                        sp[:], lhsT=vr_sb[:, m:m + 1], rhs=tfs[m][:],
                        start=(m == 0), stop=(m == MC - 1),
                    )
                srow = smp.tile([1, SBLK], F32, tag="srow")
                nc.scalar.activation(srow[:], sp[:], AF.Identity)
                nc.sync.dma_start(
                    sq[bq_local(b), sb * SBLK:(sb + 1) * SBLK], srow[:]
                )

            def bq_local(b):
                return slice(b % QB, b % QB + 1)

            def emit_softmax(q):
                sq = sq_tiles.pop(q)
                nm = smp.tile([QB, 1], F32, tag="nm")
                nc.vector.reduce_max(nm[:], sq[:], axis=AX.X, negate=True)
                aq = smp.tile([QB, S], F32, tag="aq")
                se = smp.tile([QB, 1], F32, tag="se")
                nc.scalar.activation(
                    aq[:], sq[:], AF.Exp, bias=nm[:], accum_out=se[:]
                )
                rv = smp.tile([QB, 1], F32, tag="rv")
                nc.vector.reciprocal(rv[:], se[:])
                aw = smp.tile([QB, S], F32, tag="aw", name=f"aw{q}")
                nc.vector.tensor_scalar_mul(aw[:], aq[:], rv[:])
                nc.sync.dma_start(attnW[q * QB:(q + 1) * QB, :], aw[:])
                for c in range(KC):
                    tp = tps.tile([128, QB], F32, tag="tp")
                    nc.tensor.transpose(
                        tp[:], aw[:, c * 128:(c + 1) * 128], eye_sb[:QB, :QB]
                    )
                    nc.scalar.activation(
                        attnT_sb[:, c, q * QB:(q + 1) * QB], tp[:], AF.Identity
                    )

            def emit_ctx(b):
                c0 = cps.tile([1, 512], F32, tag="ctx", name="c0")
                c1 = cps.tile([1, 512], F32, tag="ctx", name="c1")
                for sc in range(KC):
                    en = enp.tile([128, U], DT, tag="en")
                    nc.sync.dma_start(
                        en[:], encN[b, sc * 128:(sc + 1) * 128, :]
                    )
                    nc.tensor.matmul(
                        c0[:], lhsT=attnT_sb[:, sc, b:b + 1],
                        rhs=en[:, 0:512],
                        start=(sc == 0), stop=(sc == KC - 1),
                    )
                    nc.tensor.matmul(
                        c1[:], lhsT=attnT_sb[:, sc, b:b + 1],
                        rhs=en[:, 512:1024],
                        start=(sc == 0), stop=(sc == KC - 1),
                    )
                crow = smp.tile([1, U], F32, tag="crow")
                nc.scalar.activation(crow[:, 0:512], c0[:], AF.Identity)
                nc.scalar.activation(crow[:, 512:1024], c1[:], AF.Identity)
                nc.sync.dma_start(ctx_sb[b:b + 1, :], crow[:])

            groups = [(b, sb) for b in range(BL) for sb in range(NSB)]
            for b, sb in groups:
                emit_score_group(b, sb)
                if sb == NSB - 1 and b % QB == QB - 1:
                    q = b // QB
                    emit_softmax(q)
                    pending_ctx.extend(range(q * QB, (q + 1) * QB))
                elif pending_ctx:
                    emit_ctx(pending_ctx.pop(0))
            for b in pending_ctx:
                emit_ctx(b)

        # ---- GRU cell + fc, unit-sharded across cores ----
        with tc.tile_pool(name="gwork", bufs=1) as gwp, \
             tc.tile_pool(name="xgps", bufs=2, space="PSUM") as xps, \
             tc.tile_pool(name="tp2ps", bufs=2, space="PSUM") as tp2, \
             tc.tile_pool(name="fcps", bufs=1, space="PSUM") as fcp:

            # AllGather context rows -> full batch-major context, transpose
            nc.sync.dma_start(cgin[:], ctx_sb[:])
            nc.gpsimd.collective_compute(
                "AllGather", mybir.AluOpType.bypass,
                replica_groups=[list(range(N_CORES))],
                ins=[cgin.opt()], outs=[cgout.opt()],
            )
            ctxF = gwp.tile([B, U], F32, tag="ctxF")
            nc.sync.dma_start(ctxF[:], cgout[:])
            ginT = gwp.tile([128, KC, B], DT, tag="ginT")
            for c in range(KC):
                tp = tp2.tile([128, B], F32, tag="tpc")
                nc.tensor.transpose(
                    tp[:], ctxF[:, c * 128:(c + 1) * 128], eye_sb[:]
                )
                nc.scalar.activation(ginT[:, c, :], tp[:], AF.Identity)

            def gate_matmul(gi, hid_sb):
                """xg = Wk_s[:,gi].T @ gin + Wr_s[:,gi].T @ hid."""
                cols = slice(gi * 128, (gi + 1) * 128)
                xp = xps.tile([128, B], F32, tag="xg", name=f"xg{gi}")
                for k in range(KC):
                    nc.tensor.matmul(
                        xp[:], lhsT=wkA[:, k, cols], rhs=ginT[:, k, :],
                        start=(k == 0), stop=False,
                    )
                nc.tensor.matmul(
                    xp[:], lhsT=wkB[:, cols], rhs=x2_sb[:],
                    start=False, stop=False,
                )
                for k in range(KC):
                    nc.tensor.matmul(
                        xp[:], lhsT=wr_sb[:, k, cols],
                        rhs=hid_sb[:, k, :],
                        start=False, stop=(k == KC - 1),
                    )
                return xp

            zc = gwp.tile([128, B], F32, tag="zc")
            rc = gwp.tile([128, B], F32, tag="rc")
            xpz = gate_matmul(0, hTf_sb)
            nc.scalar.activation(zc[:], xpz[:], AF.Sigmoid, bias=bgs_sb[:, 0:1])
            xpr = gate_matmul(1, hTf_sb)
            nc.scalar.activation(rc[:], xpr[:], AF.Sigmoid, bias=bgs_sb[:, 1:2])

            # r*h slice -> AllGather -> full rh in [unit, batch] layout
            rhs_t = gwp.tile([128, B], DT, tag="rhs")
            nc.vector.tensor_mul(rhs_t[:], rc[:], hfs_sb[:])
            nc.sync.dma_start(rgin[:], rhs_t[:])
            nc.gpsimd.collective_compute(
                "AllGather", mybir.AluOpType.bypass,
                replica_groups=[list(range(N_CORES))],
                ins=[rgin.opt()], outs=[rgout.opt()],
            )
            rhF = gwp.tile([128, KC, B], DT, tag="rhF")
            nc.sync.dma_start(rhF[:], rgout.rearrange("(k p) b -> p k b", p=128))

            hhc = gwp.tile([128, B], F32, tag="hhc")
            xph = gate_matmul(2, rhF)
            nc.scalar.activation(hhc[:], xph[:], AF.Tanh, bias=bgs_sb[:, 2:3])

            # state = hh + z * (h - hh)
            dc = gwp.tile([128, B], F32, tag="dc")
            sc = gwp.tile([128, B], F32, tag="sc")
            nc.vector.tensor_sub(dc[:], hfs_sb[:], hhc[:])
            nc.vector.tensor_mul(dc[:], zc[:], dc[:])
            nc.vector.tensor_add(sc[:], hhc[:], dc[:])
            nc.sync.dma_start(stateT_s[:], sc[:])

            # fc partial on this core's unit slice, AllReduce, + bias
            scd = gwp.tile([128, B], DT, tag="scd")
            nc.scalar.activation(scd[:], sc[:], AF.Identity)
            fcpt = fcp.tile([IN, B], F32, tag="fc")
            nc.tensor.matmul(fcpt[:], lhsT=wfc_sb[:], rhs=scd[:],
                             start=True, stop=True)
            fcs = gwp.tile([IN, B], F32, tag="fcs")
            nc.scalar.activation(fcs[:], fcpt[:], AF.Identity)
            nc.sync.dma_start(fcin[:], fcs[:])
            nc.gpsimd.collective_compute(
                "AllReduce", mybir.AluOpType.add,
                replica_groups=[list(range(N_CORES))],
                ins=[fcin.opt()], outs=[fcout.opt()],
            )
            fres = gwp.tile([IN, B], F32, tag="fres")
            nc.sync.dma_start(fres[:], fcout[:])
            ot = gwp.tile([IN, B], F32, tag="ot")
            nc.scalar.activation(ot[:], fres[:], AF.Identity, bias=bfc_sb[:])
            nc.sync.dma_start(outT, ot[:])

    nc.compile()
    return nc


_NC_CACHE = {}
LAST_RESULT = None


def _get_nc():
    if MODE not in _NC_CACHE:
        _NC_CACHE[MODE] = build_nc()
    return _NC_CACHE[MODE]


def kernel(x, hidden, enc_output, W1, b1, W2, b2, V, bV, Wk, Wr, bg, Wfc, bfc):
    global LAST_RESULT
    x = np.asarray(x, dtype=np.float32)
    hidden = np.asarray(hidden, dtype=np.float32)
    enc_output = np.asarray(enc_output, dtype=np.float32)
    W1 = np.asarray(W1, dtype=np.float32)
    b1 = np.asarray(b1, dtype=np.float32)
    W2 = np.asarray(W2, dtype=np.float32)
    b2 = np.asarray(b2, dtype=np.float32)
    V = np.asarray(V, dtype=np.float32)
    Wk = np.asarray(Wk, dtype=np.float32)
    Wr = np.asarray(Wr, dtype=np.float32)
    bg = np.asarray(bg, dtype=np.float32)
    Wfc = np.asarray(Wfc, dtype=np.float32)
    bfc = np.asarray(bfc, dtype=np.float32)

    if MODE == "bf16":
        import ml_dtypes
        dtm_np = ml_dtypes.bfloat16
    else:
        dtm_np = np.float32

    hT_all = np.ascontiguousarray(hidden.T)              # (U, B)
    x2T_all = np.ascontiguousarray(x[:, 0, :].T)         # (IN, B)

    # replicated weights / layouts
    W1m = np.ascontiguousarray(W1).astype(dtm_np)
    W2m = np.ascontiguousarray(W2).astype(dtm_np)
    Vr = np.ascontiguousarray(V[:, 0].reshape(MC, 128).T).astype(dtm_np)
    b12r = np.ascontiguousarray((b1 + b2).reshape(MC, 128).T)
    eye = np.eye(128, dtype=np.float32)
    bfc_r = bfc.reshape(IN, 1)

    in_maps = []
    for c in range(N_CORES):
        bs = slice(c * BL, (c + 1) * BL)
        us = slice(c * 128, (c + 1) * 128)
        enc_c = enc_output[bs]
        gcols = np.r_[np.arange(c * 128, (c + 1) * 128),
                      np.arange(U + c * 128, U + (c + 1) * 128),
                      np.arange(2 * U + c * 128, 2 * U + (c + 1) * 128)]
        m = {
            "encT": np.ascontiguousarray(enc_c.transpose(0, 2, 1)).astype(dtm_np),
            "encN": np.ascontiguousarray(enc_c).astype(dtm_np),
            "W1": W1m,
            "W2": W2m,
            "hTq": np.ascontiguousarray(hT_all[:, bs]).astype(dtm_np),
            "hTfull": hT_all.astype(dtm_np),
            "hf_s": np.ascontiguousarray(hT_all[us, :]),
            "Vr": Vr,
            "b12r": b12r,
            "eye128": eye,
            "x2Tf": x2T_all.astype(dtm_np),
            "Wk_s": np.ascontiguousarray(Wk[:, gcols]).astype(dtm_np),
            "Wr_s": np.ascontiguousarray(Wr[:, gcols]).astype(dtm_np),
            "bg_s": np.ascontiguousarray(
                np.stack([bg[c * 128:(c + 1) * 128],
                          bg[U + c * 128:U + (c + 1) * 128],
                          bg[2 * U + c * 128:2 * U + (c + 1) * 128]], axis=1)),
            "Wfc_s": np.ascontiguousarray(Wfc[us, :]).astype(dtm_np),
            "bfcr": bfc_r,
        }
        in_maps.append(m)

    nc = _get_nc()
    res = run_bass_kernel_spmd(nc, in_maps, list(range(N_CORES)))
    LAST_RESULT = res

    out = np.ascontiguousarray(res.results[0]["outT"].T)  # (B, IN)
    state = np.empty((B, U), np.float32)
    attn = np.empty((B, S, 1), np.float32)
    for c in range(N_CORES):
        bs = slice(c * BL, (c + 1) * BL)
        us = slice(c * 128, (c + 1) * 128)
        r = res.results[c]
        state[:, us] = r["stateT_s"].T
        attn[bs] = r["attnW"][:, :, None]
    return out, state, attn


# revision 11
# speedup vs baseline: 1.0133x; 1.0133x over previous
"""Bahdanau-attention GRU decoder step on 8 Trainium2 NeuronCores.

Data-parallel over batch for attention (16 batches/core, enc_output shipped
in transposed + natural layouts so every DMA is contiguous); the tiny GRU/fc
tail is unit-sharded across cores (weights sliced 8x on host) with per-quad
context AllGathers (overlapped with the next quad's scores), one rh
AllGather and one fc AllReduce. Heavy matmuls run as float32r (fp32
storage, full-rate PE). Context matmuls are software-pipelined between the
score groups of the following quad to keep the PE HAM-warm. Batch columns
on the GRU side follow the gather order (q, core, i); the host permutes
hidden/x inputs and un-permutes state/out outputs.
"""

from contextlib import ExitStack

import numpy as np

import concourse.bacc as bacc
import concourse.bass as bass
import concourse.mybir as mybir
import concourse.tile as tile
from concourse.bass_utils import run_bass_kernel_spmd

N_CORES = 8
B, S, U, IN = 128, 1024, 1024, 64
BL = B // N_CORES  # batches per core
KC = U // 128      # contraction chunks
MC = U // 128      # dec-unit chunks
SBLK = 512         # moving-dim block over S
NSB = S // SBLK
QB = 4             # batches per softmax/context quad
NQ = BL // QB      # quads per core
USL = 3 * 128      # per-core gate-unit slice (z|r|h cols of this core's units)

MODE = "f32r"      # "f32" | "f32r" | "bf16"

F32 = mybir.dt.float32
AF = mybir.ActivationFunctionType
AX = mybir.AxisListType

# device batch-column order on the GRU side: j = 32q + 4c + i <-> b = 16c+4q+i
PERM = np.array([16 * c + 4 * q + i
                 for q in range(NQ) for c in range(N_CORES) for i in range(QB)])


def _dt():
    return {
        "f32": F32,
        "f32r": mybir.dt.float32r,
        "bf16": mybir.dt.bfloat16,
    }[MODE]


def build_nc():
    nc = bacc.Bacc(
        "TRN2", target_bir_lowering=False, debug=False, num_devices=N_CORES
    )
    DT = _dt()

    # ---- DRAM I/O ----
    encT = nc.dram_tensor("encT", [BL, U, S], DT, kind="ExternalInput").ap()
    encN = nc.dram_tensor("encN", [BL, S, U], DT, kind="ExternalInput").ap()
    W1 = nc.dram_tensor("W1", [U, U], DT, kind="ExternalInput").ap()
    W2 = nc.dram_tensor("W2", [U, U], DT, kind="ExternalInput").ap()
    hTq = nc.dram_tensor("hTq", [U, BL], DT, kind="ExternalInput").ap()
    hTfull = nc.dram_tensor("hTfull", [U, B], DT, kind="ExternalInput").ap()
    hf_s = nc.dram_tensor("hf_s", [128, B], F32, kind="ExternalInput").ap()
    Vr = nc.dram_tensor("Vr", [128, MC], DT, kind="ExternalInput").ap()
    b12r = nc.dram_tensor("b12r", [128, MC], F32, kind="ExternalInput").ap()
    eye128 = nc.dram_tensor("eye128", [128, 128], F32, kind="ExternalInput").ap()
    x2Tf = nc.dram_tensor("x2Tf", [IN, B], DT, kind="ExternalInput").ap()
    Wk_s = nc.dram_tensor("Wk_s", [U + IN, USL], DT, kind="ExternalInput").ap()
    Wr_s = nc.dram_tensor("Wr_s", [U, USL], DT, kind="ExternalInput").ap()
    bg_s = nc.dram_tensor("bg_s", [128, 3], F32, kind="ExternalInput").ap()
    Wfc_s = nc.dram_tensor("Wfc_s", [128, IN], DT, kind="ExternalInput").ap()
    bfcr = nc.dram_tensor("bfcr", [IN, 1], F32, kind="ExternalInput").ap()

    attnW = nc.dram_tensor("attnW", [BL, S], F32, kind="ExternalOutput").ap()
    stateT_s = nc.dram_tensor("stateT_s", [128, B], F32, kind="ExternalOutput").ap()
    outT = nc.dram_tensor("outT", [IN, B], F32, kind="ExternalOutput").ap()

    with tile.TileContext(nc) as tc, ExitStack() as ctx:
        cpool = ctx.enter_context(tc.tile_pool(name="const", bufs=1))
        w1p = ctx.enter_context(tc.tile_pool(name="w1", bufs=1))
        gcp = ctx.enter_context(tc.tile_pool(name="gconst", bufs=1))
        dramp = ctx.enter_context(tc.tile_pool(name="dram", bufs=1, space="DRAM"))

        # ---- W1 as per-m column blocks so the first matmul starts early ----
        W1r = W1.rearrange("(k p) u -> p k u", p=128)
        w1m = []
        for m in range(MC):
            t = w1p.tile([128, KC, 128], DT, tag=f"w1_{m}", name=f"w1m{m}")
            nc.sync.dma_start(t[:], W1r[:, :, m * 128:(m + 1) * 128])
            w1m.append(t)

        # ---- constants / small inputs (needed early) ----
        eye_sb = cpool.tile([128, 128], F32, tag="eye")
        nc.sync.dma_start(eye_sb[:], eye128)
        vr_sb = cpool.tile([128, MC], DT, tag="vr")
        nc.sync.dma_start(vr_sb[:], Vr)
        b12_sb = cpool.tile([128, MC], F32, tag="b12")
        nc.sync.dma_start(b12_sb[:], b12r)
        hTq_sb = cpool.tile([128, KC, BL], DT, tag="hTq")
        nc.sync.dma_start(hTq_sb[:], hTq.rearrange("(k p) b -> p k b", p=128))
        qT_sb = cpool.tile([128, MC, BL], F32, tag="qT")
        attnT_sb = cpool.tile([128, KC, BL], DT, tag="attnT")
        ctx_sb = cpool.tile([BL, U], F32, tag="ctxrow")

        # GRU tiles (allocated up front; DMAs emitted mid-loop)
        wkA = gcp.tile([128, KC, USL], DT, tag="wkA")
        wkB = gcp.tile([IN, USL], DT, tag="wkB")
        wr_sb = gcp.tile([128, KC, USL], DT, tag="wr")
        hTf_sb = gcp.tile([128, KC, B], DT, tag="hTfull")
        hfs_sb = gcp.tile([128, B], F32, tag="hfs")
        x2_sb = gcp.tile([IN, B], DT, tag="x2")
        bgs_sb = gcp.tile([128, 3], F32, tag="bgs")
        wfc_sb = gcp.tile([128, IN], DT, tag="wfc")
        bfc_sb = gcp.tile([IN, 1], F32, tag="bfc")
        ginT = gcp.tile([128, KC, B], DT, tag="ginT")

        def emit_gru_prefetch():
            nc.sync.dma_start(
                wkA[:], Wk_s[0:U, :].rearrange("(k p) j -> p k j", p=128)
            )
            nc.sync.dma_start(wkB[:], Wk_s[U:U + IN, :])
            nc.sync.dma_start(wr_sb[:], Wr_s.rearrange("(k p) j -> p k j", p=128))
            nc.sync.dma_start(
                hTf_sb[:], hTfull.rearrange("(k p) b -> p k b", p=128)
            )
            nc.sync.dma_start(hfs_sb[:], hf_s)
            nc.sync.dma_start(x2_sb[:], x2Tf)
            nc.sync.dma_start(bgs_sb[:], bg_s)
            nc.sync.dma_start(wfc_sb[:], Wfc_s)
            nc.sync.dma_start(bfc_sb[:], bfcr)

        # collective bounce buffers (internal DRAM)
        cgin = [dramp.tile([QB, U], F32, tag=f"cgin{q}", name=f"cgin{q}")
                for q in range(NQ)]
        cgout = [dramp.tile([N_CORES * QB, U], F32, tag=f"cgout{q}",
                            name=f"cgout{q}") for q in range(NQ)]
        rgin = dramp.tile([128, B], DT, tag="rgin")
        rgout = dramp.tile([U, B], DT, tag="rgout")
        fcin = dramp.tile([IN, B], F32, tag="fcin")
        fcout = dramp.tile([IN, B], F32, tag="fcout")

        with tc.tile_pool(name="encTp", bufs=2) as etp, \
             tc.tile_pool(name="w2s", bufs=2) as w2p, \
             tc.tile_pool(name="tf", bufs=12) as tfp, \
             tc.tile_pool(name="encNp", bufs=2) as enp, \
             tc.tile_pool(name="sm", bufs=2) as smp, \
             tc.tile_pool(name="featps", bufs=3, space="PSUM") as fps, \
             tc.tile_pool(name="scoreps", bufs=2, space="PSUM") as sps, \
             tc.tile_pool(name="ctxps", bufs=2, space="PSUM") as cps, \
             tc.tile_pool(name="tpps", bufs=1, space="PSUM") as tps:

            # ---- phase 0: q = hidden @ W2 (+b1+b2), transposed into qT ----
            def emit_phase0():
                q0 = cps.tile([BL, 512], F32, tag="ctx", name="q0")
                q1 = cps.tile([BL, 512], F32, tag="ctx", name="q1")
                for k in range(KC):
                    w2t = w2p.tile([128, U], DT, tag="w2t")
                    nc.sync.dma_start(w2t[:], W2[k * 128:(k + 1) * 128, :])
                    nc.tensor.matmul(
                        q0[:], lhsT=hTq_sb[:, k, :], rhs=w2t[:, 0:512],
                        start=(k == 0), stop=(k == KC - 1),
                    )
                    nc.tensor.matmul(
                        q1[:], lhsT=hTq_sb[:, k, :], rhs=w2t[:, 512:1024],
                        start=(k == 0), stop=(k == KC - 1),
                    )
                q_sb = smp.tile([BL, U], F32, tag="q_sb")
                nc.scalar.activation(q_sb[:, 0:512], q0[:], AF.Identity)
                nc.scalar.activation(q_sb[:, 512:1024], q1[:], AF.Identity)
                for m in range(MC):
                    tp = tps.tile([128, BL], F32, tag="tp", name=f"qtp{m}")
                    nc.tensor.transpose(
                        tp[:], q_sb[:, m * 128:(m + 1) * 128],
                        eye_sb[:BL, :BL],
                    )
                    nc.scalar.activation(
                        qT_sb[:, m, :], tp[:], AF.Identity,
                        bias=b12_sb[:, m:m + 1],
                    )

            sq_tiles = {}
            pending_ctx = []
            ctx_done = [0] * NQ

            def emit_score_group(b, sb):
                q = b // QB
                if (b % QB, sb) == (0, 0):
                    sq_tiles[q] = smp.tile([QB, S], F32, tag="sq",
                                           name=f"sq{q}")
                sq = sq_tiles[q]
                et = etp.tile([128, KC, SBLK], DT, tag="et")
                src = encT[b].rearrange("(k p) s -> p k s", p=128)
                for k in range(KC):
                    nc.sync.dma_start(
                        et[:, k, :],
                        src[:, k, sb * SBLK:(sb + 1) * SBLK],
                    )
                tfs = []
                for m in range(MC):
                    fp = fps.tile([128, SBLK], F32, tag="fp")
                    for k in range(KC):
                        nc.tensor.matmul(
                            fp[:],
                            lhsT=w1m[m][:, k, :],
                            rhs=et[:, k, :],
                            start=(k == 0),
                            stop=(k == KC - 1),
                        )
                    tf = tfp.tile([128, SBLK], DT, tag="tf")
                    nc.scalar.activation(
                        tf[:], fp[:], AF.Tanh, bias=qT_sb[:, m, b:b + 1]
                    )
                    tfs.append(tf)
                sp = sps.tile([1, SBLK], F32, tag="sp")
                for m in range(MC):
                    nc.tensor.matmul(
                        sp[:], lhsT=vr_sb[:, m:m + 1], rhs=tfs[m][:],
                        start=(m == 0), stop=(m == MC - 1),
                    )
                srow = smp.tile([1, SBLK], F32, tag="srow")
                nc.scalar.activation(srow[:], sp[:], AF.Identity)
                nc.sync.dma_start(
                    sq[b % QB:b % QB + 1, sb * SBLK:(sb + 1) * SBLK], srow[:]
                )

            def emit_softmax(q):
                sq = sq_tiles.pop(q)
                nm = smp.tile([QB, 1], F32, tag="nm")
                nc.vector.reduce_max(nm[:], sq[:], axis=AX.X, negate=True)
                aq = smp.tile([QB, S], F32, tag="aq")
                se = smp.tile([QB, 1], F32, tag="se")
                nc.scalar.activation(
                    aq[:], sq[:], AF.Exp, bias=nm[:], accum_out=se[:]
                )
                rv = smp.tile([QB, 1], F32, tag="rv")
                nc.vector.reciprocal(rv[:], se[:])
                aw = smp.tile([QB, S], F32, tag="aw", name=f"aw{q}")
                nc.vector.tensor_scalar_mul(aw[:], aq[:], rv[:])
                nc.sync.dma_start(attnW[q * QB:(q + 1) * QB, :], aw[:])
                for c in range(KC):
                    tp = tps.tile([128, QB], F32, tag="tp")
                    nc.tensor.transpose(
                        tp[:], aw[:, c * 128:(c + 1) * 128], eye_sb[:QB, :QB]
                    )
                    nc.scalar.activation(
                        attnT_sb[:, c, q * QB:(q + 1) * QB], tp[:], AF.Identity
                    )

            def emit_ctx(b):
                c0 = cps.tile([1, 512], F32, tag="ctx", name="c0")
                c1 = cps.tile([1, 512], F32, tag="ctx", name="c1")
                for sc in range(KC):
                    en = enp.tile([128, U], DT, tag="en")
                    nc.sync.dma_start(
                        en[:], encN[b, sc * 128:(sc + 1) * 128, :]
                    )
                    nc.tensor.matmul(
                        c0[:], lhsT=attnT_sb[:, sc, b:b + 1],
                        rhs=en[:, 0:512],
                        start=(sc == 0), stop=(sc == KC - 1),
                    )
                    nc.tensor.matmul(
                        c1[:], lhsT=attnT_sb[:, sc, b:b + 1],
                        rhs=en[:, 512:1024],
                        start=(sc == 0), stop=(sc == KC - 1),
                    )
                crow = smp.tile([1, U], F32, tag="crow")
                nc.scalar.activation(crow[:, 0:512], c0[:], AF.Identity)
                nc.scalar.activation(crow[:, 512:1024], c1[:], AF.Identity)
                nc.sync.dma_start(ctx_sb[b:b + 1, :], crow[:])
                # when a quad's 4 contexts are all written, gather + transpose
                q = b // QB
                ctx_done[q] += 1
                if ctx_done[q] == QB:
                    emit_ctx_gather(q)

            def emit_ctx_gather(q):
                nc.sync.dma_start(cgin[q][:], ctx_sb[q * QB:(q + 1) * QB, :])
                nc.gpsimd.collective_compute(
                    "AllGather", mybir.AluOpType.bypass,
                    replica_groups=[list(range(N_CORES))],
                    ins=[cgin[q].opt()], outs=[cgout[q].opt()],
                )
                ctxF = smp.tile([N_CORES * QB, U], F32, tag="ctxF",
                                name=f"ctxF{q}")
                nc.sync.dma_start(ctxF[:], cgout[q][:])
                for c in range(KC):
                    tp = tps.tile([128, N_CORES * QB], F32, tag="tp",
                                  name=f"gtp{q}_{c}")
                    nc.tensor.transpose(
                        tp[:], ctxF[:, c * 128:(c + 1) * 128],
                        eye_sb[:N_CORES * QB, :N_CORES * QB],
                    )
                    nc.scalar.activation(
                        ginT[:, c, q * 32:(q + 1) * 32], tp[:], AF.Identity
                    )

            # ---- emission schedule ----
            emit_phase0()
            groups = [(b, sb) for b in range(BL) for sb in range(NSB)]
            for gi, (b, sb) in enumerate(groups):
                emit_score_group(b, sb)
                if gi == 6:
                    emit_gru_prefetch()
                if sb == NSB - 1 and b % QB == QB - 1:
                    q = b // QB
                    emit_softmax(q)
                    pending_ctx.extend(range(q * QB, (q + 1) * QB))
                elif pending_ctx:
                    emit_ctx(pending_ctx.pop(0))
            for b in pending_ctx:
                emit_ctx(b)

        # ---- GRU cell + fc, unit-sharded across cores ----
        with tc.tile_pool(name="gwork", bufs=1) as gwp, \
             tc.tile_pool(name="xgps", bufs=2, space="PSUM") as xps, \
             tc.tile_pool(name="fcps", bufs=1, space="PSUM") as fcp:

            def gate_matmul(gi, hid_sb):
                cols = slice(gi * 128, (gi + 1) * 128)
                xp = xps.tile([128, B], F32, tag="xg", name=f"xg{gi}")
                for k in range(KC):
                    nc.tensor.matmul(
                        xp[:], lhsT=wkA[:, k, cols], rhs=ginT[:, k, :],
                        start=(k == 0), stop=False,
                    )
                nc.tensor.matmul(
                    xp[:], lhsT=wkB[:, cols], rhs=x2_sb[:],
                    start=False, stop=False,
                )
                for k in range(KC):
                    nc.tensor.matmul(
                        xp[:], lhsT=wr_sb[:, k, cols],
                        rhs=hid_sb[:, k, :],
                        start=False, stop=(k == KC - 1),
                    )
                return xp

            zc = gwp.tile([128, B], F32, tag="zc")
            rc = gwp.tile([128, B], F32, tag="rc")
            xpz = gate_matmul(0, hTf_sb)
            nc.scalar.activation(zc[:], xpz[:], AF.Sigmoid, bias=bgs_sb[:, 0:1])
            xpr = gate_matmul(1, hTf_sb)
            nc.scalar.activation(rc[:], xpr[:], AF.Sigmoid, bias=bgs_sb[:, 1:2])

            # r*h slice -> AllGather -> full rh in [unit, batch] layout
            rhs_t = gwp.tile([128, B], DT, tag="rhs")
            nc.vector.tensor_mul(rhs_t[:], rc[:], hfs_sb[:])
            nc.sync.dma_start(rgin[:], rhs_t[:])
            nc.gpsimd.collective_compute(
                "AllGather", mybir.AluOpType.bypass,
                replica_groups=[list(range(N_CORES))],
                ins=[rgin.opt()], outs=[rgout.opt()],
            )
            rhF = gwp.tile([128, KC, B], DT, tag="rhF")
            nc.sync.dma_start(rhF[:], rgout.rearrange("(k p) b -> p k b", p=128))

            hhc = gwp.tile([128, B], F32, tag="hhc")
            xph = gate_matmul(2, rhF)
            nc.scalar.activation(hhc[:], xph[:], AF.Tanh, bias=bgs_sb[:, 2:3])

            # state = hh + z * (h - hh)
            dc = gwp.tile([128, B], F32, tag="dc")
            sc = gwp.tile([128, B], F32, tag="sc")
            nc.vector.tensor_sub(dc[:], hfs_sb[:], hhc[:])
            nc.vector.tensor_mul(dc[:], zc[:], dc[:])
            nc.vector.tensor_add(sc[:], hhc[:], dc[:])
            nc.sync.dma_start(stateT_s[:], sc[:])

            # fc partial on this core's unit slice, AllReduce, + bias
            scd = gwp.tile([128, B], DT, tag="scd")
            nc.scalar.activation(scd[:], sc[:], AF.Identity)
            fcpt = fcp.tile([IN, B], F32, tag="fc")
            nc.tensor.matmul(fcpt[:], lhsT=wfc_sb[:], rhs=scd[:],
                             start=True, stop=True)
            fcs = gwp.tile([IN, B], F32, tag="fcs")
            nc.scalar.activation(fcs[:], fcpt[:], AF.Identity)
            nc.sync.dma_start(fcin[:], fcs[:])
            nc.gpsimd.collective_compute(
                "AllReduce", mybir.AluOpType.add,
                replica_groups=[list(range(N_CORES))],
                ins=[fcin.opt()], outs=[fcout.opt()],
            )
            fres = gwp.tile([IN, B], F32, tag="fres")
            nc.sync.dma_start(fres[:], fcout[:])
            ot = gwp.tile([IN, B], F32, tag="ot")
            nc.scalar.activation(ot[:], fres[:], AF.Identity, bias=bfc_sb[:])
            nc.sync.dma_start(outT, ot[:])

    nc.compile()
    return nc


_NC_CACHE = {}
LAST_RESULT = None


def _get_nc():
    if MODE not in _NC_CACHE:
        _NC_CACHE[MODE] = build_nc()
    return _NC_CACHE[MODE]


def kernel(x, hidden, enc_output, W1, b1, W2, b2, V, bV, Wk, Wr, bg, Wfc, bfc):
    global LAST_RESULT
    x = np.asarray(x, dtype=np.float32)
    hidden = np.asarray(hidden, dtype=np.float32)
    enc_output = np.asarray(enc_output, dtype=np.float32)
    W1 = np.asarray(W1, dtype=np.float32)
    b1 = np.asarray(b1, dtype=np.float32)
    W2 = np.asarray(W2, dtype=np.float32)
    b2 = np.asarray(b2, dtype=np.float32)
    V = np.asarray(V, dtype=np.float32)
    Wk = np.asarray(Wk, dtype=np.float32)
    Wr = np.asarray(Wr, dtype=np.float32)
    bg = np.asarray(bg, dtype=np.float32)
    Wfc = np.asarray(Wfc, dtype=np.float32)
    bfc = np.asarray(bfc, dtype=np.float32)

    if MODE == "bf16":
        import ml_dtypes
        dtm_np = ml_dtypes.bfloat16
    else:
        dtm_np = np.float32

    hT_all = np.ascontiguousarray(hidden.T)              # (U, B)
    x2T_all = np.ascontiguousarray(x[:, 0, :].T)         # (IN, B)
    hT_perm = np.ascontiguousarray(hT_all[:, PERM])
    x2T_perm = np.ascontiguousarray(x2T_all[:, PERM])

    # replicated weights / layouts
    W1m = np.ascontiguousarray(W1).astype(dtm_np)
    W2m = np.ascontiguousarray(W2).astype(dtm_np)
    Vr = np.ascontiguousarray(V[:, 0].reshape(MC, 128).T).astype(dtm_np)
    b12r = np.ascontiguousarray((b1 + b2).reshape(MC, 128).T)
    eye = np.eye(128, dtype=np.float32)
    bfc_r = bfc.reshape(IN, 1)

    in_maps = []
    for c in range(N_CORES):
        bs = slice(c * BL, (c + 1) * BL)
        us = slice(c * 128, (c + 1) * 128)
        enc_c = enc_output[bs]
        gcols = np.r_[np.arange(c * 128, (c + 1) * 128),
                      np.arange(U + c * 128, U + (c + 1) * 128),
                      np.arange(2 * U + c * 128, 2 * U + (c + 1) * 128)]
        m = {
            "encT": np.ascontiguousarray(enc_c.transpose(0, 2, 1)).astype(dtm_np),
            "encN": np.ascontiguousarray(enc_c).astype(dtm_np),
            "W1": W1m,
            "W2": W2m,
            "hTq": np.ascontiguousarray(hT_all[:, bs]).astype(dtm_np),
            "hTfull": hT_perm.astype(dtm_np),
            "hf_s": np.ascontiguousarray(hT_perm[us, :]),
            "Vr": Vr,
            "b12r": b12r,
            "eye128": eye,
            "x2Tf": x2T_perm.astype(dtm_np),
            "Wk_s": np.ascontiguousarray(Wk[:, gcols]).astype(dtm_np),
            "Wr_s": np.ascontiguousarray(Wr[:, gcols]).astype(dtm_np),
            "bg_s": np.ascontiguousarray(
                np.stack([bg[c * 128:(c + 1) * 128],
                          bg[U + c * 128:U + (c + 1) * 128],
                          bg[2 * U + c * 128:2 * U + (c + 1) * 128]], axis=1)),
            "Wfc_s": np.ascontiguousarray(Wfc[us, :]).astype(dtm_np),
            "bfcr": bfc_r,
        }
        in_maps.append(m)

    nc = _get_nc()
    res = run_bass_kernel_spmd(nc, in_maps, list(range(N_CORES)))
    LAST_RESULT = res

    out = np.empty((B, IN), np.float32)
    out[PERM] = res.results[0]["outT"].T
    state = np.empty((B, U), np.float32)
    attn = np.empty((B, S, 1), np.float32)
    for c in range(N_CORES):
        bs = slice(c * BL, (c + 1) * BL)
        us = slice(c * 128, (c + 1) * 128)
        r = res.results[c]
        state[PERM, us.start:us.stop] = r["stateT_s"].T
        attn[bs] = r["attnW"][:, :, None]
    return out, state, attn


# revision 12
# speedup vs baseline: 1.0431x; 1.0294x over previous
"""Bahdanau-attention GRU decoder step on 8 Trainium2 NeuronCores.

Data-parallel over batch for attention (16 batches/core, enc_output shipped
in transposed + natural layouts so every DMA is contiguous); the tiny GRU/fc
tail is unit-sharded across cores (weights sliced 8x on host) with per-group
context AllGathers (overlapped with the next group's scores), one rh
AllGather and one fc AllReduce. Heavy matmuls run as float32r (fp32
storage, full-rate PE). Context matmuls are software-pipelined between the
score groups that follow, and batch groups are sized [6,6,3,1] so almost no
context/gather work trails the last score group. Batch columns on the GRU
side follow the gather order; the host permutes hidden/x inputs and
un-permutes state/out outputs.
"""

from contextlib import ExitStack

import numpy as np

import concourse.bacc as bacc
import concourse.bass as bass
import concourse.mybir as mybir
import concourse.tile as tile
from concourse.bass_utils import run_bass_kernel_spmd

N_CORES = 8
B, S, U, IN = 128, 1024, 1024, 64
BL = B // N_CORES  # batches per core
KC = U // 128      # contraction chunks
MC = U // 128      # dec-unit chunks
SBLK = 512         # moving-dim block over S
NSB = S // SBLK
USL = 3 * 128      # per-core gate-unit slice (z|r|h cols of this core's units)

# batch groups per core (softmax/context granularity); last groups small so
# their context+gather work doesn't trail the final score groups
GSIZES = [6, 6, 3, 1]
GSTARTS = np.cumsum([0] + GSIZES).tolist()
NG = len(GSIZES)

MODE = "f32r"      # "f32" | "f32r" | "bf16"

F32 = mybir.dt.float32
AF = mybir.ActivationFunctionType
AX = mybir.AxisListType

# device batch-column order on the GRU side: group-major, then core, then
# local index: col = 8*GSTARTS[g] + GSIZES[g]*c + i <-> batch 16c+GSTARTS[g]+i
PERM = np.array([16 * c + GSTARTS[g] + i
                 for g in range(NG) for c in range(N_CORES)
                 for i in range(GSIZES[g])])


def _dt():
    return {
        "f32": F32,
        "f32r": mybir.dt.float32r,
        "bf16": mybir.dt.bfloat16,
    }[MODE]


def build_nc():
    nc = bacc.Bacc(
        "TRN2", target_bir_lowering=False, debug=False, num_devices=N_CORES
    )
    DT = _dt()

    # ---- DRAM I/O ----
    encT = nc.dram_tensor("encT", [BL, U, S], DT, kind="ExternalInput").ap()
    encN = nc.dram_tensor("encN", [BL, S, U], DT, kind="ExternalInput").ap()
    W1 = nc.dram_tensor("W1", [U, U], DT, kind="ExternalInput").ap()
    W2 = nc.dram_tensor("W2", [U, U], DT, kind="ExternalInput").ap()
    hTq = nc.dram_tensor("hTq", [U, BL], DT, kind="ExternalInput").ap()
    hTfull = nc.dram_tensor("hTfull", [U, B], DT, kind="ExternalInput").ap()
    hf_s = nc.dram_tensor("hf_s", [128, B], F32, kind="ExternalInput").ap()
    Vr = nc.dram_tensor("Vr", [128, MC], DT, kind="ExternalInput").ap()
    b12r = nc.dram_tensor("b12r", [128, MC], F32, kind="ExternalInput").ap()
    eye128 = nc.dram_tensor("eye128", [128, 128], F32, kind="ExternalInput").ap()
    x2Tf = nc.dram_tensor("x2Tf", [IN, B], DT, kind="ExternalInput").ap()
    Wk_s = nc.dram_tensor("Wk_s", [U + IN, USL], DT, kind="ExternalInput").ap()
    Wr_s = nc.dram_tensor("Wr_s", [U, USL], DT, kind="ExternalInput").ap()
    bg_s = nc.dram_tensor("bg_s", [128, 3], F32, kind="ExternalInput").ap()
    Wfc_s = nc.dram_tensor("Wfc_s", [128, IN], DT, kind="ExternalInput").ap()
    bfcr = nc.dram_tensor("bfcr", [IN, 1], F32, kind="ExternalInput").ap()

    attnW = nc.dram_tensor("attnW", [BL, S], F32, kind="ExternalOutput").ap()
    stateT_s = nc.dram_tensor("stateT_s", [128, B], F32, kind="ExternalOutput").ap()
    outT = nc.dram_tensor("outT", [IN, B], F32, kind="ExternalOutput").ap()

    with tile.TileContext(nc) as tc, ExitStack() as ctx:
        cpool = ctx.enter_context(tc.tile_pool(name="const", bufs=1))
        w1p = ctx.enter_context(tc.tile_pool(name="w1", bufs=1))
        gcp = ctx.enter_context(tc.tile_pool(name="gconst", bufs=1))
        dramp = ctx.enter_context(tc.tile_pool(name="dram", bufs=1, space="DRAM"))

        W1r = W1.rearrange("(k p) u -> p k u", p=128)
        w1m = [w1p.tile([128, KC, 128], DT, tag=f"w1_{m}", name=f"w1m{m}")
               for m in range(MC)]

        # constants needed early (small)
        eye_sb = cpool.tile([128, 128], F32, tag="eye")
        vr_sb = cpool.tile([128, MC], DT, tag="vr")
        b12_sb = cpool.tile([128, MC], F32, tag="b12")
        hTq_sb = cpool.tile([128, KC, BL], DT, tag="hTq")
        qT_sb = cpool.tile([128, MC, BL], F32, tag="qT")
        attnT_sb = cpool.tile([128, KC, BL], DT, tag="attnT")
        ctx_sb = cpool.tile([BL, U], F32, tag="ctxrow")

        # GRU tiles (allocated up front; DMAs emitted mid-loop)
        wkA = gcp.tile([128, KC, USL], DT, tag="wkA")
        wkB = gcp.tile([IN, USL], DT, tag="wkB")
        wr_sb = gcp.tile([128, KC, USL], DT, tag="wr")
        hTf_sb = gcp.tile([128, KC, B], DT, tag="hTfull")
        hfs_sb = gcp.tile([128, B], F32, tag="hfs")
        x2_sb = gcp.tile([IN, B], DT, tag="x2")
        bgs_sb = gcp.tile([128, 3], F32, tag="bgs")
        wfc_sb = gcp.tile([128, IN], DT, tag="wfc")
        bfc_sb = gcp.tile([IN, 1], F32, tag="bfc")
        ginT = gcp.tile([128, KC, B], DT, tag="ginT")

        def emit_gru_prefetch():
            nc.sync.dma_start(
                wkA[:], Wk_s[0:U, :].rearrange("(k p) j -> p k j", p=128)
            )
            nc.sync.dma_start(wkB[:], Wk_s[U:U + IN, :])
            nc.sync.dma_start(wr_sb[:], Wr_s.rearrange("(k p) j -> p k j", p=128))
            nc.sync.dma_start(
                hTf_sb[:], hTfull.rearrange("(k p) b -> p k b", p=128)
            )
            nc.sync.dma_start(hfs_sb[:], hf_s)
            nc.sync.dma_start(x2_sb[:], x2Tf)
            nc.sync.dma_start(bgs_sb[:], bg_s)
            nc.sync.dma_start(wfc_sb[:], Wfc_s)
            nc.sync.dma_start(bfc_sb[:], bfcr)

        # collective bounce buffers (internal DRAM)
        cgin = [dramp.tile([GSIZES[g], U], F32, tag=f"cgin{g}",
                           name=f"cgin{g}") for g in range(NG)]
        cgout = [dramp.tile([N_CORES * GSIZES[g], U], F32, tag=f"cgout{g}",
                            name=f"cgout{g}") for g in range(NG)]
        rgin = dramp.tile([128, B], DT, tag="rgin")
        rgout = dramp.tile([U, B], DT, tag="rgout")
        fcin = dramp.tile([IN, B], F32, tag="fcin")
        fcout = dramp.tile([IN, B], F32, tag="fcout")

        with tc.tile_pool(name="encTp", bufs=2) as etp, \
             tc.tile_pool(name="w2s", bufs=2) as w2p, \
             tc.tile_pool(name="tf", bufs=12) as tfp, \
             tc.tile_pool(name="encNp", bufs=2) as enp, \
             tc.tile_pool(name="sm", bufs=2) as smp, \
             tc.tile_pool(name="featps", bufs=3, space="PSUM") as fps, \
             tc.tile_pool(name="scoreps", bufs=2, space="PSUM") as sps, \
             tc.tile_pool(name="ctxps", bufs=2, space="PSUM") as cps, \
             tc.tile_pool(name="tpps", bufs=1, space="PSUM") as tps:

            def emit_phase0():
                q0 = cps.tile([BL, 512], F32, tag="ctx", name="q0")
                q1 = cps.tile([BL, 512], F32, tag="ctx", name="q1")
                for k in range(KC):
                    w2t = w2p.tile([128, U], DT, tag="w2t")
                    nc.sync.dma_start(w2t[:], W2[k * 128:(k + 1) * 128, :])
                    nc.tensor.matmul(
                        q0[:], lhsT=hTq_sb[:, k, :], rhs=w2t[:, 0:512],
                        start=(k == 0), stop=(k == KC - 1),
                    )
                    nc.tensor.matmul(
                        q1[:], lhsT=hTq_sb[:, k, :], rhs=w2t[:, 512:1024],
                        start=(k == 0), stop=(k == KC - 1),
                    )
                q_sb = smp.tile([BL, U], F32, tag="q_sb")
                nc.scalar.activation(q_sb[:, 0:512], q0[:], AF.Identity)
                nc.scalar.activation(q_sb[:, 512:1024], q1[:], AF.Identity)
                for m in range(MC):
                    tp = tps.tile([128, BL], F32, tag="tp", name=f"qtp{m}")
                    nc.tensor.transpose(
                        tp[:], q_sb[:, m * 128:(m + 1) * 128],
                        eye_sb[:BL, :BL],
                    )
                    nc.scalar.activation(
                        qT_sb[:, m, :], tp[:], AF.Identity,
                        bias=b12_sb[:, m:m + 1],
                    )

            et_cache = {}

            def emit_et(b, sb):
                et = etp.tile([128, KC, SBLK], DT, tag="et",
                              name=f"et{b}_{sb}")
                src = encT[b].rearrange("(k p) s -> p k s", p=128)
                for k in range(KC):
                    nc.sync.dma_start(
                        et[:, k, :], src[:, k, sb * SBLK:(sb + 1) * SBLK]
                    )
                et_cache[(b, sb)] = et

            sq_tiles = {}

            def emit_score_group(b, sb):
                g = next(i for i in range(NG)
                         if GSTARTS[i] <= b < GSTARTS[i + 1])
                n = GSIZES[g]
                if (b == GSTARTS[g]) and sb == 0:
                    sq_tiles[g] = smp.tile([n, S], F32, tag="sq",
                                           name=f"sq{g}")
                sq = sq_tiles[g]
                if (b, sb) not in et_cache:
                    emit_et(b, sb)
                et = et_cache.pop((b, sb))
                tfs = []
                for m in range(MC):
                    fp = fps.tile([128, SBLK], F32, tag="fp")
                    for k in range(KC):
                        nc.tensor.matmul(
                            fp[:], lhsT=w1m[m][:, k, :], rhs=et[:, k, :],
                            start=(k == 0), stop=(k == KC - 1),
                        )
                    tf = tfp.tile([128, SBLK], DT, tag="tf")
                    nc.scalar.activation(
                        tf[:], fp[:], AF.Tanh, bias=qT_sb[:, m, b:b + 1]
                    )
                    tfs.append(tf)
                sp = sps.tile([1, SBLK], F32, tag="sp")
                for m in range(MC):
                    nc.tensor.matmul(
                        sp[:], lhsT=vr_sb[:, m:m + 1], rhs=tfs[m][:],
                        start=(m == 0), stop=(m == MC - 1),
                    )
                srow = smp.tile([1, SBLK], F32, tag="srow")
                nc.scalar.activation(srow[:], sp[:], AF.Identity)
                bl = b - GSTARTS[g]
                nc.sync.dma_start(
                    sq[bl:bl + 1, sb * SBLK:(sb + 1) * SBLK], srow[:]
                )

            def emit_softmax(g):
                n = GSIZES[g]
                b0 = GSTARTS[g]
                sq = sq_tiles.pop(g)
                nm = smp.tile([n, 1], F32, tag="nm", name=f"nm{g}")
                nc.vector.reduce_max(nm[:], sq[:], axis=AX.X, negate=True)
                aq = smp.tile([n, S], F32, tag="aq", name=f"aq{g}")
                se = smp.tile([n, 1], F32, tag="se", name=f"se{g}")
                nc.scalar.activation(
                    aq[:], sq[:], AF.Exp, bias=nm[:], accum_out=se[:]
                )
                rv = smp.tile([n, 1], F32, tag="rv", name=f"rv{g}")
                nc.vector.reciprocal(rv[:], se[:])
                aw = smp.tile([n, S], F32, tag="aw", name=f"aw{g}")
                nc.vector.tensor_scalar_mul(aw[:], aq[:], rv[:])
                nc.sync.dma_start(attnW[b0:b0 + n, :], aw[:])
                for c in range(KC):
                    tp = tps.tile([128, n], F32, tag="tp", name=f"atp{g}_{c}")
                    nc.tensor.transpose(
                        tp[:], aw[:, c * 128:(c + 1) * 128], eye_sb[:n, :n]
                    )
                    nc.scalar.activation(
                        attnT_sb[:, c, b0:b0 + n], tp[:], AF.Identity
                    )

            def emit_ctx(b):
                c0 = cps.tile([1, 512], F32, tag="ctx", name="c0")
                c1 = cps.tile([1, 512], F32, tag="ctx", name="c1")
                for scn in range(KC):
                    en = enp.tile([128, U], DT, tag="en")
                    nc.sync.dma_start(
                        en[:], encN[b, scn * 128:(scn + 1) * 128, :]
                    )
                    nc.tensor.matmul(
                        c0[:], lhsT=attnT_sb[:, scn, b:b + 1],
                        rhs=en[:, 0:512],
                        start=(scn == 0), stop=(scn == KC - 1),
                    )
                    nc.tensor.matmul(
                        c1[:], lhsT=attnT_sb[:, scn, b:b + 1],
                        rhs=en[:, 512:1024],
                        start=(scn == 0), stop=(scn == KC - 1),
                    )
                crow = smp.tile([1, U], F32, tag="crow")
                nc.scalar.activation(crow[:, 0:512], c0[:], AF.Identity)
                nc.scalar.activation(crow[:, 512:1024], c1[:], AF.Identity)
                nc.sync.dma_start(ctx_sb[b:b + 1, :], crow[:])

            def emit_ctx_gather(g):
                n = GSIZES[g]
                b0 = GSTARTS[g]
                nc.sync.dma_start(cgin[g][:], ctx_sb[b0:b0 + n, :])
                nc.gpsimd.collective_compute(
                    "AllGather", mybir.AluOpType.bypass,
                    replica_groups=[list(range(N_CORES))],
                    ins=[cgin[g].opt()], outs=[cgout[g].opt()],
                )

            def emit_gather_transpose(g):
                n = GSIZES[g]
                nn_ = N_CORES * n
                col0 = 8 * GSTARTS[g]
                ctxF = smp.tile([nn_, U], F32, tag="ctxF", name=f"ctxF{g}")
                nc.sync.dma_start(ctxF[:], cgout[g][:])
                for c in range(KC):
                    tp = tps.tile([128, nn_], F32, tag="tp",
                                  name=f"gtp{g}_{c}")
                    nc.tensor.transpose(
                        tp[:], ctxF[:, c * 128:(c + 1) * 128],
                        eye_sb[:nn_, :nn_],
                    )
                    nc.scalar.activation(
                        ginT[:, c, col0:col0 + nn_], tp[:], AF.Identity
                    )

            # ---- emission schedule ----
            # early DMAs in priority order: first W1 block, phase-0 weights,
            # first two groups' enc tiles, the rest of W1, then constants.
            nc.sync.dma_start(eye_sb[:], eye128)
            nc.sync.dma_start(hTq_sb[:], hTq.rearrange("(k p) b -> p k b", p=128))
            nc.sync.dma_start(w1m[0][:], W1r[:, :, 0:128])
            emit_phase0()
            emit_et(0, 0)
            emit_et(0, 1)
            for m in range(1, MC):
                nc.sync.dma_start(
                    w1m[m][:], W1r[:, :, m * 128:(m + 1) * 128]
                )
            nc.sync.dma_start(vr_sb[:], Vr)
            nc.sync.dma_start(b12_sb[:], b12r)

            groups = [(b, sb) for b in range(BL) for sb in range(NSB)]
            pending_ctx = []
            deferred_tp = []  # (emit_at_gi, g)
            gdone = 0
            for gi, (b, sb) in enumerate(groups):
                emit_score_group(b, sb)
                if gi == 6:
                    emit_gru_prefetch()
                while deferred_tp and deferred_tp[0][0] <= gi:
                    emit_gather_transpose(deferred_tp.pop(0)[1])
                g = next(i for i in range(NG)
                         if GSTARTS[i] <= b < GSTARTS[i + 1])
                if sb == NSB - 1 and b == GSTARTS[g + 1] - 1:
                    emit_softmax(g)
                    pending_ctx.extend(range(GSTARTS[g], GSTARTS[g + 1]))
                else:
                    npop = 1
                    if len(pending_ctx) > (len(groups) - gi - 1):
                        npop = 2
                    for _ in range(npop):
                        if pending_ctx:
                            bb = pending_ctx.pop(0)
                            emit_ctx(bb)
                            gq = next(i for i in range(NG)
                                      if GSTARTS[i] <= bb < GSTARTS[i + 1])
                            if bb == GSTARTS[gq + 1] - 1:
                                emit_ctx_gather(gq)
                                deferred_tp.append((gi + 2, gq))
            for bb in pending_ctx:
                emit_ctx(bb)
                gq = next(i for i in range(NG)
                          if GSTARTS[i] <= bb < GSTARTS[i + 1])
                if bb == GSTARTS[gq + 1] - 1:
                    emit_ctx_gather(gq)
                    deferred_tp.append((10 ** 9, gq))
            for _, gq in deferred_tp:
                emit_gather_transpose(gq)

        # ---- GRU cell + fc, unit-sharded across cores ----
        with tc.tile_pool(name="gwork", bufs=1) as gwp, \
             tc.tile_pool(name="xgps", bufs=2, space="PSUM") as xps, \
             tc.tile_pool(name="fcps", bufs=1, space="PSUM") as fcp:

            def gate_matmul(gi_, hid_sb):
                cols = slice(gi_ * 128, (gi_ + 1) * 128)
                xp = xps.tile([128, B], F32, tag="xg", name=f"xg{gi_}")
                for k in range(KC):
                    nc.tensor.matmul(
                        xp[:], lhsT=wkA[:, k, cols], rhs=ginT[:, k, :],
                        start=(k == 0), stop=False,
                    )
                nc.tensor.matmul(
                    xp[:], lhsT=wkB[:, cols], rhs=x2_sb[:],
                    start=False, stop=False,
                )
                for k in range(KC):
                    nc.tensor.matmul(
                        xp[:], lhsT=wr_sb[:, k, cols],
                        rhs=hid_sb[:, k, :],
                        start=False, stop=(k == KC - 1),
                    )
                return xp

            zc = gwp.tile([128, B], F32, tag="zc")
            rc = gwp.tile([128, B], F32, tag="rc")
            xpz = gate_matmul(0, hTf_sb)
            nc.scalar.activation(zc[:], xpz[:], AF.Sigmoid, bias=bgs_sb[:, 0:1])
            xpr = gate_matmul(1, hTf_sb)
            nc.scalar.activation(rc[:], xpr[:], AF.Sigmoid, bias=bgs_sb[:, 1:2])

            rhs_t = gwp.tile([128, B], DT, tag="rhs")
            nc.vector.tensor_mul(rhs_t[:], rc[:], hfs_sb[:])
            nc.sync.dma_start(rgin[:], rhs_t[:])
            nc.gpsimd.collective_compute(
                "AllGather", mybir.AluOpType.bypass,
                replica_groups=[list(range(N_CORES))],
                ins=[rgin.opt()], outs=[rgout.opt()],
            )
            rhF = gwp.tile([128, KC, B], DT, tag="rhF")
            nc.sync.dma_start(rhF[:], rgout.rearrange("(k p) b -> p k b", p=128))

            hhc = gwp.tile([128, B], F32, tag="hhc")
            xph = gate_matmul(2, rhF)
            nc.scalar.activation(hhc[:], xph[:], AF.Tanh, bias=bgs_sb[:, 2:3])

            dc = gwp.tile([128, B], F32, tag="dc")
            sc = gwp.tile([128, B], F32, tag="sc")
            nc.vector.tensor_sub(dc[:], hfs_sb[:], hhc[:])
            nc.vector.tensor_mul(dc[:], zc[:], dc[:])
            nc.vector.tensor_add(sc[:], hhc[:], dc[:])
            nc.sync.dma_start(stateT_s[:], sc[:])

            scd = gwp.tile([128, B], DT, tag="scd")
            nc.scalar.activation(scd[:], sc[:], AF.Identity)
            fcpt = fcp.tile([IN, B], F32, tag="fc")
            nc.tensor.matmul(fcpt[:], lhsT=wfc_sb[:], rhs=scd[:],
                             start=True, stop=True)
            fcs = gwp.tile([IN, B], F32, tag="fcs")
            nc.scalar.activation(fcs[:], fcpt[:], AF.Identity)
            nc.sync.dma_start(fcin[:], fcs[:])
            nc.gpsimd.collective_compute(
                "AllReduce", mybir.AluOpType.add,
                replica_groups=[list(range(N_CORES))],
                ins=[fcin.opt()], outs=[fcout.opt()],
            )
            fres = gwp.tile([IN, B], F32, tag="fres")
            nc.sync.dma_start(fres[:], fcout[:])
            ot = gwp.tile([IN, B], F32, tag="ot")
            nc.scalar.activation(ot[:], fres[:], AF.Identity, bias=bfc_sb[:])
            nc.sync.dma_start(outT, ot[:])

    nc.compile()
    return nc


_NC_CACHE = {}
LAST_RESULT = None


def _get_nc():
    if MODE not in _NC_CACHE:
        _NC_CACHE[MODE] = build_nc()
    return _NC_CACHE[MODE]


def kernel(x, hidden, enc_output, W1, b1, W2, b2, V, bV, Wk, Wr, bg, Wfc, bfc):
    global LAST_RESULT
    x = np.asarray(x, dtype=np.float32)
    hidden = np.asarray(hidden, dtype=np.float32)
    enc_output = np.asarray(enc_output, dtype=np.float32)
    W1 = np.asarray(W1, dtype=np.float32)
    b1 = np.asarray(b1, dtype=np.float32)
    W2 = np.asarray(W2, dtype=np.float32)
    b2 = np.asarray(b2, dtype=np.float32)
    V = np.asarray(V, dtype=np.float32)
    Wk = np.asarray(Wk, dtype=np.float32)
    Wr = np.asarray(Wr, dtype=np.float32)
    bg = np.asarray(bg, dtype=np.float32)
    Wfc = np.asarray(Wfc, dtype=np.float32)
    bfc = np.asarray(bfc, dtype=np.float32)

    if MODE == "bf16":
        import ml_dtypes
        dtm_np = ml_dtypes.bfloat16
    else:
        dtm_np = np.float32

    hT_all = np.ascontiguousarray(hidden.T)              # (U, B)
    x2T_all = np.ascontiguousarray(x[:, 0, :].T)         # (IN, B)
    hT_perm = np.ascontiguousarray(hT_all[:, PERM])
    x2T_perm = np.ascontiguousarray(x2T_all[:, PERM])

    W1m = np.ascontiguousarray(W1).astype(dtm_np)
    W2m = np.ascontiguousarray(W2).astype(dtm_np)
    Vr = np.ascontiguousarray(V[:, 0].reshape(MC, 128).T).astype(dtm_np)
    b12r = np.ascontiguousarray((b1 + b2).reshape(MC, 128).T)
    eye = np.eye(128, dtype=np.float32)
    bfc_r = bfc.reshape(IN, 1)

    in_maps = []
    for c in range(N_CORES):
        bs = slice(c * BL, (c + 1) * BL)
        us = slice(c * 128, (c + 1) * 128)
        enc_c = enc_output[bs]
        gcols = np.r_[np.arange(c * 128, (c + 1) * 128),
                      np.arange(U + c * 128, U + (c + 1) * 128),
                      np.arange(2 * U + c * 128, 2 * U + (c + 1) * 128)]
        m = {
            "encT": np.ascontiguousarray(enc_c.transpose(0, 2, 1)).astype(dtm_np),
            "encN": np.ascontiguousarray(enc_c).astype(dtm_np),
            "W1": W1m,
            "W2": W2m,
            "hTq": np.ascontiguousarray(hT_all[:, bs]).astype(dtm_np),
            "hTfull": hT_perm.astype(dtm_np),
            "hf_s": np.ascontiguousarray(hT_perm[us, :]),
            "Vr": Vr,
            "b12r": b12r,
            "eye128": eye,
            "x2Tf": x2T_perm.astype(dtm_np),
            "Wk_s": np.ascontiguousarray(Wk[:, gcols]).astype(dtm_np),
            "Wr_s": np.ascontiguousarray(Wr[:, gcols]).astype(dtm_np),
            "bg_s": np.ascontiguousarray(
                np.stack([bg[c * 128:(c + 1) * 128],
                          bg[U + c * 128:U + (c + 1) * 128],
                          bg[2 * U + c * 128:2 * U + (c + 1) * 128]], axis=1)),
            "Wfc_s": np.ascontiguousarray(Wfc[us, :]).astype(dtm_np),
            "bfcr": bfc_r,
        }
        in_maps.append(m)

    nc = _get_nc()
    res = run_bass_kernel_spmd(nc, in_maps, list(range(N_CORES)))
    LAST_RESULT = res

    out = np.empty((B, IN), np.float32)
    out[PERM] = res.results[0]["outT"].T
    state = np.empty((B, U), np.float32)
    attn = np.empty((B, S, 1), np.float32)
    for c in range(N_CORES):
        bs = slice(c * BL, (c + 1) * BL)
        us = slice(c * 128, (c + 1) * 128)
        r = res.results[c]
        state[PERM, us.start:us.stop] = r["stateT_s"].T
        attn[bs] = r["attnW"][:, :, None]
    return out, state, attn
